# revision 22
# baseline (speedup 1.0000x reference)
"""CenterLoss kernel for Trainium2, data-parallel across 8 NeuronCores.

Math: the reference masks the full [B, C] squared-distance matrix with
one_hot(labels) and clamps to [1e-12, 1e12] before summing.  The mask keeps
only distmat[i, labels[i]]; every other entry becomes clip(0) = 1e-12, so

    loss = ( sum_i ||x_i - c_{l_i}||^2 + B*(C-1)*1e-12 ) / B

Per core (B/8 = 2048 rows), raw bass pipeline, p-major row layout
(shard row 16*p + r lives at partition p, chunk r; r in [0,16)).

The critical path is Q7 descriptor emission for the 2048-row center gather:
~9.1 ns/row on the resident indirect-DMA ucode, serialized on the Pool
engine.  Measured alternatives are all worse:
  - InstDMAGatherAnt (dma_gather): 10.3 ns/idx emission PLUS ~8.6 us of
    one-time 'mlp' Q7 library load inside the measured window; crashes
    outright at num_idxs=2048 (1024 max).
  - 2D offset APs on indirect_dma_start: the HW ucode reads ONE index per
    partition per instruction (sim diverges; OOB-crashes the exec unit).
  - tensor_tensor_reduce: crashes the DVE on this runtime build.
  - fp8: halves DMA drain (not the bottleneck) but drops DVE tensor_tensor
    from 2x to 1x mode -> slower compute, worse accuracy.  bf16 kept.
So: 16 back-to-back indirect_dma_start's (128 rows each, ~1.1-1.2 us),
everything else hidden under that stream:
  - labels load issued from the Pool engine itself (SWDGE) as its first
    instruction -> gather stream starts ~1 us earlier than via sync.
  - dynamic_dma_scratch_size=64KB quadruples the SWDGE descriptor ring to
    reduce Q7 ring-reclaim stalls while x-loads keep the SDMA engines busy.
  - vector/scalar consume at 2-chunk granularity on pair semaphores;
    acc columns 0-5 are stored early to hide the final DMA receipt.
"""

import sys
from contextlib import ExitStack

import ml_dtypes
import numpy as np

try:
    import concourse.bass  # noqa: F401
except ImportError:
    sys.path.insert(0, "/opt/trn_rl_repo")

import concourse.bass as bass
import concourse.mybir as mybir
from concourse.bacc import Bacc
from concourse.bass_utils import run_bass_kernel_spmd

B, C, D = 16384, 1000, 512
N_CORES = 8
B_SHARD = B // N_CORES  # 2048
P = 128
NCHUNK = B_SHARD // P  # 16 chunks, chunk r = rows {16p + r}
NPAIR = NCHUNK // 2  # 8 compute pairs
NFULL = NPAIR - 1  # pairs 0..6 full-size; chunks 14/15 run singly (tail)
NCOL = NFULL + 2  # 9 accumulator columns
CLAMP_MIN = 1e-12

import os

SEMHALF = os.environ.get("SEMHALF", "0") == "1"

_NC_CACHE = {}


def build_nc():
    nc = Bacc(dynamic_dma_scratch_size=2**16)
    f32 = mybir.dt.float32
    bf16 = mybir.dt.bfloat16
    x_d = nc.declare_dram_parameter("x", [B_SHARD, D], bf16, isOutput=False)
    lbl_d = nc.declare_dram_parameter(
        "labels", [P, NCHUNK], mybir.dt.int32, isOutput=False
    )
    cen_d = nc.declare_dram_parameter("centers", [C, D], bf16, isOutput=False)
    out_d = nc.declare_dram_parameter("out", [P, NCOL], f32, isOutput=True)

    x_r = x_d.rearrange("(p r) d -> p r d", p=P)  # [128, 16, 512]

    with ExitStack() as ctx:
        x_sb = ctx.enter_context(nc.sbuf_tensor("x_sb", [P, NCHUNK, D], bf16))
        g_sb = ctx.enter_context(nc.sbuf_tensor("g_sb", [P, NCHUNK, D], bf16))
        diff_sb = ctx.enter_context(nc.sbuf_tensor("diff_sb", [P, 3, 2, D], bf16))
        sq_sb = ctx.enter_context(nc.sbuf_tensor("sq_sb", [P, 2, D], bf16))
        lbl_sb = ctx.enter_context(
            nc.sbuf_tensor("lbl_sb", [P, NCHUNK], mybir.dt.int32)
        )
        acc_sb = ctx.enter_context(nc.sbuf_tensor("acc_sb", [P, NCOL], f32))

        # all SWDGE gathers are sem-confirmed by consumers, so the expensive
        # Pool dge_drain in the block epilogue is redundant
        block = ctx.enter_context(nc.Block(no_gpsimd_drain=True))
        ls = ctx.enter_context(nc.semaphore("ls"))
        xs = [ctx.enter_context(nc.semaphore(f"xs{q}")) for q in range(2)]
        gs = [ctx.enter_context(nc.semaphore(f"gs{k}")) for k in range(NFULL)]
        gt = [ctx.enter_context(nc.semaphore(f"gt{j}")) for j in range(2)]
        vs = ctx.enter_context(nc.semaphore("vs"))
        ss = ctx.enter_context(nc.semaphore("ss"))
        os_ = ctx.enter_context(nc.semaphore("os"))

        @block.sync
        def _(sync):
            # labels first: they drain ahead of x on the same HWDGE queue
            # (FIFO per queue), gating the gather stream as early as possible
            sync.dma_start(out=lbl_sb[:], in_=lbl_d[:]).then_inc(ls, 16)
            for q in range(2):
                sync.dma_start(
                    out=x_sb[:, q * 8 : (q + 1) * 8, :],
                    in_=x_r[:, q * 8 : (q + 1) * 8, :],
                ).then_inc(xs[q], 16)

        @block.gpsimd
        def _(gpsimd):
            gpsimd.wait_ge(ls, 16)
            for r in range(NCHUNK):
                dma = gpsimd.indirect_dma_start(
                    out=g_sb[:, r, :],
                    out_offset=None,
                    in_=cen_d[:],
                    in_offset=bass.IndirectOffsetOnAxis(
                        ap=lbl_sb[:, r : r + 1], axis=0
                    ),
                )
                if r >= 14:
                    # last two chunks signal individually: shortens the
                    # tail (compute consumes them singly)
                    dma.then_inc(gt[r - 14], 16)
                elif SEMHALF:
                    # sem only on the odd chunk of each pair: qPoolDynamic
                    # is FIFO per engine ring, so chunk 2k's descriptors
                    # complete before chunk 2k+1's final descriptor on
                    # every engine.  (Invisible to the sim race detector —
                    # HW-only experiment.)
                    if r % 2 == 1:
                        dma.then_inc(gs[r // 2], 16)
                else:
                    dma.then_inc(gs[r // 2], 16)

        @block.vector
        def _(vector):
            for k in range(NFULL):
                vector.wait_ge(xs[k // 4], 16)
                # both chunks of the pair landed
                vector.wait_ge(gs[k], 16 if SEMHALF else 32)
                if k >= 2:
                    vector.wait_ge(ss, k - 1)  # WAR: scalar done with diff slot
                vector.tensor_tensor(
                    out=diff_sb[:, k % 2, :, :],
                    in0=x_sb[:, 2 * k : 2 * k + 2, :],
                    in1=g_sb[:, 2 * k : 2 * k + 2, :],
                    op=mybir.AluOpType.subtract,
                ).then_inc(vs, 1)
            for j in range(2):  # chunks 14, 15 singly into slot 2
                vector.wait_ge(xs[1], 16)
                vector.wait_ge(gt[j], 16)
                vector.tensor_tensor(
                    out=diff_sb[:, 2, j : j + 1, :],
                    in0=x_sb[:, 14 + j : 15 + j, :],
                    in1=g_sb[:, 14 + j : 15 + j, :],
                    op=mybir.AluOpType.subtract,
                ).then_inc(vs, 1)

        @block.scalar
        def _(scalar):
            for k in range(NFULL):
                scalar.wait_ge(vs, k + 1)
                if k:
                    # shared sq dummy: same-engine WAW, free wait for the
                    # race detector
                    scalar.wait_ge(ss, k)
                scalar.activation(
                    out=sq_sb[:, :, :],
                    in_=diff_sb[:, k % 2, :, :],
                    func=mybir.ActivationFunctionType.Square,
                    accum_out=acc_sb[:, k : k + 1],
                ).then_inc(ss, 1)
                if k == NFULL - 2:
                    # early store of the first 5 columns hides most of the
                    # final DMA's completion receipt behind the last pairs.
                    # ss fires on ACTIVATION_READ_ACCUMULATOR completion, so
                    # this wait orders the store after the accum writes (the
                    # DMA trigger otherwise races the accumulator read-out).
                    scalar.wait_ge(ss, NFULL - 1)
                    scalar.dma_start(
                        out=out_d[:, : NFULL - 1], in_=acc_sb[:, : NFULL - 1]
                    ).then_inc(os_, 16)
            for j in range(2):  # chunks 14, 15 singly (short tail ops)
                scalar.wait_ge(vs, NFULL + j + 1)
                scalar.wait_ge(ss, NFULL + j)
                scalar.activation(
                    out=sq_sb[:, j : j + 1, :],
                    in_=diff_sb[:, 2, j : j + 1, :],
                    func=mybir.ActivationFunctionType.Square,
                    accum_out=acc_sb[:, NFULL + j : NFULL + j + 1],
                ).then_inc(ss, 1)
            scalar.wait_ge(ss, NFULL + 2)
            scalar.dma_start(
                out=out_d[:, NFULL - 1 :], in_=acc_sb[:, NFULL - 1 :]
            ).then_inc(os_, 16)
            scalar.wait_ge(os_, 32)

    nc.finalize()
    return nc


def _get_nc():
    if "nc" not in _NC_CACHE:
        _NC_CACHE["nc"] = build_nc()
    return _NC_CACHE["nc"]


def kernel(x, labels, centers, _trace=False):
    x = np.asarray(x, dtype=np.float32).astype(ml_dtypes.bfloat16)
    centers = np.asarray(centers, dtype=np.float32).astype(ml_dtypes.bfloat16)
    labels_i = np.asarray(labels).astype(np.int32)

    in_maps = []
    for i in range(N_CORES):
        xs_ = np.ascontiguousarray(x[i * B_SHARD : (i + 1) * B_SHARD])
        ls_ = labels_i[i * B_SHARD : (i + 1) * B_SHARD]
        in_maps.append(
            {
                "x": xs_,
                # row 16p + r at [p, r]
                "labels": np.ascontiguousarray(ls_.reshape(P, NCHUNK)),
                "centers": centers,
            }
        )

    nc = _get_nc()
    res = run_bass_kernel_spmd(nc, in_maps, list(range(N_CORES)), trace=_trace)
    partials = np.stack([r["out"] for r in res.results])  # [8, 128, NCOL]
    total = np.sum(partials.astype(np.float64))
    total += B * (C - 1) * CLAMP_MIN
    loss = np.float32(total / B)
    if _trace:
        return np.asarray(loss), res
    return np.asarray(loss)


# revision 26
# speedup vs baseline: 1.0323x; 1.0323x over previous
"""CenterLoss kernel for Trainium2, data-parallel across 8 NeuronCores.

Math: the reference masks the full [B, C] squared-distance matrix with
one_hot(labels) and clamps to [1e-12, 1e12] before summing.  The mask keeps
only distmat[i, labels[i]]; every other entry becomes clip(0) = 1e-12, so

    loss = ( sum_i ||x_i - c_{l_i}||^2 + B*(C-1)*1e-12 ) / B

Per core (B/8 = 2048 rows), raw bass pipeline, p-major row layout
(shard row 16*p + r lives at partition p, chunk r; r in [0,16)).

The critical path is Q7 descriptor emission for the 2048-row center gather:
~8.6 ns/row plus ~310 ns/instruction dispatch on the resident indirect-DMA
ucode, serialized on the Pool engine (16 x ~1.41 us = 22.5 us).  Measured
dead ends (all verified on hardware this session):
  - InstDMAGatherAnt (dma_gather, 'mlp' Q7 library): 10.3 ns/idx emission
    PLUS ~8.6 us of one-time library load inside the measured window;
    crashes the exec unit outright at num_idxs=2048 (1024 verified max);
    transpose mode crashes even at 1024.
  - Batched (2D) offset APs on indirect_dma_start: the HW ucode reads ONE
    index per partition per instruction and streams the rest contiguously
    (CoreSim diverges from HW here; larger spans OOB-crash the exec unit).
    128 rows/instruction is a hard cap.
  - tensor_tensor_reduce: crashes the DVE on this runtime build.
  - fp8 x/centers: halves DMA drain (not the bottleneck) but drops DVE
    tensor_tensor from 2x to 1x perf mode -> slower compute, worse
    accuracy.  bf16 kept.
  - Merging gather sems into thresholds of one sem: unsafe (the 16 SDMA
    engines drain their rings independently, so sem >= 16k does not imply
    the first k DMAs landed) and walrus rejects unsem'd DMAs anyway.
  - A fixed ~8 us EVENT_SEMAPHORE epilogue (block barrier + teardown)
    exists in every run regardless of sem count or instruction count.
Everything else hides under the gather stream:
  - labels load issued first on the sync queue: HWDGE rings are FIFO per
    queue, so the 8KB labels tile drains ahead of the 2MB x load and the
    gather stream starts ~3 us into the window.
  - dynamic_dma_scratch_size=64KB quadruples the SWDGE descriptor ring:
    keeps the 16 emissions at a stable ~1.1 us (the 16KB default ring
    showed ring-reclaim drift up to 1.47 us under concurrent x traffic).
  - vector/scalar consume at 2-chunk granularity on pair semaphores
    (small DVE ops also minimize SBUF-port contention with the Q7's
    descriptor rings in partitions 0-31); chunks 14/15 gather and compute
    singly (426/720 ns ops) to shorten the tail.
  - acc columns 0-5 are stored early to hide the final DMA's completion
    receipt behind the last pairs.
"""

import sys
from contextlib import ExitStack

import ml_dtypes
import numpy as np

try:
    import concourse.bass  # noqa: F401
except ImportError:
    sys.path.insert(0, "/opt/trn_rl_repo")

import concourse.bass as bass
import concourse.mybir as mybir
from concourse.bacc import Bacc
from concourse.bass_utils import run_bass_kernel_spmd

B, C, D = 16384, 1000, 512
N_CORES = 8
B_SHARD = B // N_CORES  # 2048
P = 128
NCHUNK = B_SHARD // P  # 16 chunks, chunk r = rows {16p + r}
NPAIR = NCHUNK // 2  # 8 compute pairs
NFULL = NPAIR - 1  # pairs 0..6 full-size; chunks 14/15 run singly (tail)
NCOL = NFULL + 2  # 9 accumulator columns
CLAMP_MIN = 1e-12

_NC_CACHE = {}


def build_nc():
    nc = Bacc(dynamic_dma_scratch_size=2**16)
    f32 = mybir.dt.float32
    bf16 = mybir.dt.bfloat16
    x_d = nc.declare_dram_parameter("x", [B_SHARD, D], bf16, isOutput=False)
    lbl_d = nc.declare_dram_parameter(
        "labels", [P, NCHUNK], mybir.dt.int32, isOutput=False
    )
    cen_d = nc.declare_dram_parameter("centers", [C, D], bf16, isOutput=False)
    out_d = nc.declare_dram_parameter("out", [P, NCOL], f32, isOutput=True)

    x_r = x_d.rearrange("(p r) d -> p r d", p=P)  # [128, 16, 512]

    with ExitStack() as ctx:
        x_sb = ctx.enter_context(nc.sbuf_tensor("x_sb", [P, NCHUNK, D], bf16))
        g_sb = ctx.enter_context(nc.sbuf_tensor("g_sb", [P, NCHUNK, D], bf16))
        diff_sb = ctx.enter_context(nc.sbuf_tensor("diff_sb", [P, 3, 2, D], bf16))
        sq_sb = ctx.enter_context(nc.sbuf_tensor("sq_sb", [P, 2, D], bf16))
        lbl_sb = ctx.enter_context(
            nc.sbuf_tensor("lbl_sb", [P, NCHUNK], mybir.dt.int32)
        )
        acc_sb = ctx.enter_context(nc.sbuf_tensor("acc_sb", [P, NCOL], f32))

        block = ctx.enter_context(nc.Block())
        ls = ctx.enter_context(nc.semaphore("ls"))
        xs = [ctx.enter_context(nc.semaphore(f"xs{q}")) for q in range(2)]
        gs = [ctx.enter_context(nc.semaphore(f"gs{k}")) for k in range(NFULL)]
        gt = [ctx.enter_context(nc.semaphore(f"gt{j}")) for j in range(2)]
        vs = ctx.enter_context(nc.semaphore("vs"))
        ss = ctx.enter_context(nc.semaphore("ss"))
        os_ = ctx.enter_context(nc.semaphore("os"))

        @block.sync
        def _(sync):
            # labels first: they drain ahead of x on the same HWDGE queue
            # (FIFO per queue), gating the gather stream as early as possible
            sync.dma_start(out=lbl_sb[:], in_=lbl_d[:]).then_inc(ls, 16)
            for q in range(2):
                sync.dma_start(
                    out=x_sb[:, q * 8 : (q + 1) * 8, :],
                    in_=x_r[:, q * 8 : (q + 1) * 8, :],
                ).then_inc(xs[q], 16)

        @block.gpsimd
        def _(gpsimd):
            gpsimd.wait_ge(ls, 16)
            for r in range(NCHUNK):
                dma = gpsimd.indirect_dma_start(
                    out=g_sb[:, r, :],
                    out_offset=None,
                    in_=cen_d[:],
                    in_offset=bass.IndirectOffsetOnAxis(
                        ap=lbl_sb[:, r : r + 1], axis=0
                    ),
                )
                if r >= 14:
                    # last two chunks signal individually: shortens the
                    # tail (compute consumes them singly)
                    dma.then_inc(gt[r - 14], 16)
                else:
                    dma.then_inc(gs[r // 2], 16)

        @block.vector
        def _(vector):
            for k in range(NFULL):
                vector.wait_ge(xs[k // 4], 16)
                vector.wait_ge(gs[k], 32)  # both chunks of the pair landed
                if k >= 2:
                    vector.wait_ge(ss, k - 1)  # WAR: scalar done with diff slot
                vector.tensor_tensor(
                    out=diff_sb[:, k % 2, :, :],
                    in0=x_sb[:, 2 * k : 2 * k + 2, :],
                    in1=g_sb[:, 2 * k : 2 * k + 2, :],
                    op=mybir.AluOpType.subtract,
                ).then_inc(vs, 1)
            for j in range(2):  # chunks 14, 15 singly into slot 2
                vector.wait_ge(xs[1], 16)
                vector.wait_ge(gt[j], 16)
                vector.tensor_tensor(
                    out=diff_sb[:, 2, j : j + 1, :],
                    in0=x_sb[:, 14 + j : 15 + j, :],
                    in1=g_sb[:, 14 + j : 15 + j, :],
                    op=mybir.AluOpType.subtract,
                ).then_inc(vs, 1)

        @block.scalar
        def _(scalar):
            for k in range(NFULL):
                scalar.wait_ge(vs, k + 1)
                if k:
                    # shared sq dummy: same-engine WAW, free wait for the
                    # race detector
                    scalar.wait_ge(ss, k)
                scalar.activation(
                    out=sq_sb[:, :, :],
                    in_=diff_sb[:, k % 2, :, :],
                    func=mybir.ActivationFunctionType.Square,
                    accum_out=acc_sb[:, k : k + 1],
                ).then_inc(ss, 1)
                if k == NFULL - 2:
                    # early store of the first 5 columns hides most of the
                    # final DMA's completion receipt behind the last pairs.
                    # ss fires on ACTIVATION_READ_ACCUMULATOR completion, so
                    # this wait orders the store after the accum writes (the
                    # DMA trigger otherwise races the accumulator read-out).
                    scalar.wait_ge(ss, NFULL - 1)
                    scalar.dma_start(
                        out=out_d[:, : NFULL - 1], in_=acc_sb[:, : NFULL - 1]
                    ).then_inc(os_, 16)
            for j in range(2):  # chunks 14, 15 singly (short tail ops)
                scalar.wait_ge(vs, NFULL + j + 1)
                scalar.wait_ge(ss, NFULL + j)
                scalar.activation(
                    out=sq_sb[:, j : j + 1, :],
                    in_=diff_sb[:, 2, j : j + 1, :],
                    func=mybir.ActivationFunctionType.Square,
                    accum_out=acc_sb[:, NFULL + j : NFULL + j + 1],
                ).then_inc(ss, 1)
            scalar.wait_ge(ss, NFULL + 2)
            scalar.dma_start(
                out=out_d[:, NFULL - 1 :], in_=acc_sb[:, NFULL - 1 :]
            ).then_inc(os_, 16)
            scalar.wait_ge(os_, 32)

    nc.finalize()
    return nc


def _get_nc():
    if "nc" not in _NC_CACHE:
        _NC_CACHE["nc"] = build_nc()
    return _NC_CACHE["nc"]


def kernel(x, labels, centers, _trace=False):
    x = np.asarray(x, dtype=np.float32).astype(ml_dtypes.bfloat16)
    centers = np.asarray(centers, dtype=np.float32).astype(ml_dtypes.bfloat16)
    labels_i = np.asarray(labels).astype(np.int32)

    in_maps = []
    for i in range(N_CORES):
        xs_ = np.ascontiguousarray(x[i * B_SHARD : (i + 1) * B_SHARD])
        ls_ = labels_i[i * B_SHARD : (i + 1) * B_SHARD]
        in_maps.append(
            {
                "x": xs_,
                # row 16p + r at [p, r]
                "labels": np.ascontiguousarray(ls_.reshape(P, NCHUNK)),
                "centers": centers,
            }
        )

    nc = _get_nc()
    res = run_bass_kernel_spmd(nc, in_maps, list(range(N_CORES)), trace=_trace)
    partials = np.stack([r["out"] for r in res.results])  # [8, 128, NCOL]
    total = np.sum(partials.astype(np.float64))
    total += B * (C - 1) * CLAMP_MIN
    loss = np.float32(total / B)
    if _trace:
        return np.asarray(loss), res
    return np.asarray(loss)


# revision 29
# speedup vs baseline: 1.0628x; 1.0295x over previous
"""CenterLoss kernel for Trainium2, data-parallel across 8 NeuronCores.

Math: the reference masks the full [B, C] squared-distance matrix with
one_hot(labels) and clamps to [1e-12, 1e12] before summing.  The mask keeps
only distmat[i, labels[i]]; every other entry becomes clip(0) = 1e-12, so

    loss = ( sum_i ||x_i - c_{l_i}||^2 + B*(C-1)*1e-12 ) / B

Per core (B/8 = 2048 rows), raw bass pipeline, p-major row layout
(shard row 16*p + r lives at partition p, chunk r; r in [0,16)).

The critical path is Q7 descriptor emission for the 2048-row center gather:
~8.6 ns/row plus ~310 ns/instruction dispatch on the resident indirect-DMA
ucode, serialized on the Pool engine (16 x ~1.41 us = 22.5 us).  Measured
dead ends (all verified on hardware this session):
  - InstDMAGatherAnt (dma_gather, 'mlp' Q7 library): 10.3 ns/idx emission
    PLUS ~8.6 us of one-time library load inside the measured window;
    crashes the exec unit outright at num_idxs=2048 (1024 verified max);
    transpose mode crashes even at 1024.
  - Batched (2D) offset APs on indirect_dma_start: the HW ucode reads ONE
    index per partition per instruction and streams the rest contiguously
    (CoreSim diverges from HW here; larger spans OOB-crash the exec unit).
    128 rows/instruction is a hard cap.
  - tensor_tensor_reduce: crashes the DVE on this runtime build.
  - fp8 x/centers: halves DMA drain (not the bottleneck) but drops DVE
    tensor_tensor from 2x to 1x perf mode -> slower compute, worse
    accuracy.  bf16 kept.
  - Merging gather sems into thresholds of one sem: unsafe (the 16 SDMA
    engines drain their rings independently, so sem >= 16k does not imply
    the first k DMAs landed) and walrus rejects unsem'd DMAs anyway.
  - A fixed ~8 us EVENT_SEMAPHORE epilogue (block barrier + teardown)
    exists in every run regardless of sem count or instruction count.
Everything else hides under the gather stream:
  - labels load issued first on the sync queue: HWDGE rings are FIFO per
    queue, so the 8KB labels tile drains ahead of the 2MB x load and the
    gather stream starts ~3 us into the window.
  - dynamic_dma_scratch_size=64KB quadruples the SWDGE descriptor ring:
    keeps the 16 emissions at a stable ~1.1 us (the 16KB default ring
    showed ring-reclaim drift up to 1.47 us under concurrent x traffic).
  - vector/scalar consume at 2-chunk granularity on pair semaphores
    (small DVE ops also minimize SBUF-port contention with the Q7's
    descriptor rings in partitions 0-31); chunks 14/15 gather and compute
    singly (426/720 ns ops) to shorten the tail.
  - acc columns 0-5 are stored early to hide the final DMA's completion
    receipt behind the last pairs.
"""

import sys
from contextlib import ExitStack

import ml_dtypes
import numpy as np

try:
    import concourse.bass  # noqa: F401
except ImportError:
    sys.path.insert(0, "/opt/trn_rl_repo")

import concourse.bass as bass
import concourse.mybir as mybir
from concourse.bacc import Bacc
from concourse.bass_utils import run_bass_kernel_spmd

B, C, D = 16384, 1000, 512
N_CORES = 8
B_SHARD = B // N_CORES  # 2048
P = 128
NCHUNK = B_SHARD // P  # 16 chunks, chunk r = rows {16p + r}
NPAIR = NCHUNK // 2  # 8 compute pairs
NFULL = NPAIR - 1  # pairs 0..6 full-size; chunks 14/15 run singly (tail)
NCOL = NFULL + 2  # 9 accumulator columns
CLAMP_MIN = 1e-12

_NC_CACHE = {}


def build_nc():
    nc = Bacc(dynamic_dma_scratch_size=2**16)
    f32 = mybir.dt.float32
    bf16 = mybir.dt.bfloat16
    x_d = nc.declare_dram_parameter("x", [B_SHARD, D], bf16, isOutput=False)
    lbl_d = nc.declare_dram_parameter(
        "labels", [P, NCHUNK], mybir.dt.int32, isOutput=False
    )
    cen_d = nc.declare_dram_parameter("centers", [C, D], bf16, isOutput=False)
    out_d = nc.declare_dram_parameter("out", [P, NCOL], f32, isOutput=True)

    x_r = x_d.rearrange("(p r) d -> p r d", p=P)  # [128, 16, 512]

    with ExitStack() as ctx:
        x_sb = ctx.enter_context(nc.sbuf_tensor("x_sb", [P, NCHUNK, D], bf16))
        g_sb = ctx.enter_context(nc.sbuf_tensor("g_sb", [P, NCHUNK, D], bf16))
        diff_sb = ctx.enter_context(nc.sbuf_tensor("diff_sb", [P, 3, 2, D], bf16))
        sq_sb = ctx.enter_context(nc.sbuf_tensor("sq_sb", [P, 2, D], bf16))
        lbl_sb = ctx.enter_context(
            nc.sbuf_tensor("lbl_sb", [P, NCHUNK], mybir.dt.int32)
        )
        acc_sb = ctx.enter_context(nc.sbuf_tensor("acc_sb", [P, NCOL], f32))

        block = ctx.enter_context(nc.Block())
        ls = ctx.enter_context(nc.semaphore("ls"))
        xs = [ctx.enter_context(nc.semaphore(f"xs{q}")) for q in range(2)]
        gs = [ctx.enter_context(nc.semaphore(f"gs{k}")) for k in range(NFULL)]
        gt = [ctx.enter_context(nc.semaphore(f"gt{j}")) for j in range(2)]
        vs = ctx.enter_context(nc.semaphore("vs"))
        ss = ctx.enter_context(nc.semaphore("ss"))
        os_ = ctx.enter_context(nc.semaphore("os"))

        @block.sync
        def _(sync):
            # labels first: they drain ahead of x on the same HWDGE queue
            # (FIFO per queue), gating the gather stream as early as possible
            sync.dma_start(out=lbl_sb[:], in_=lbl_d[:]).then_inc(ls, 16)
            for q in range(2):
                sync.dma_start(
                    out=x_sb[:, q * 8 : (q + 1) * 8, :],
                    in_=x_r[:, q * 8 : (q + 1) * 8, :],
                ).then_inc(xs[q], 16)

        @block.gpsimd
        def _(gpsimd):
            gpsimd.wait_ge(ls, 16)
            for r in range(NCHUNK):
                dma = gpsimd.indirect_dma_start(
                    out=g_sb[:, r, :],
                    out_offset=None,
                    in_=cen_d[:],
                    in_offset=bass.IndirectOffsetOnAxis(
                        ap=lbl_sb[:, r : r + 1], axis=0
                    ),
                )
                if r >= 14:
                    # last two chunks signal individually: shortens the
                    # tail (compute consumes them singly)
                    dma.then_inc(gt[r - 14], 16)
                else:
                    dma.then_inc(gs[r // 2], 16)

        @block.vector
        def _(vector):
            for k in range(NFULL):
                vector.wait_ge(xs[k // 4], 16)
                vector.wait_ge(gs[k], 32)  # both chunks of the pair landed
                if k >= 2:
                    vector.wait_ge(ss, k - 1)  # WAR: scalar done with diff slot
                vector.tensor_tensor(
                    out=diff_sb[:, k % 2, :, :],
                    in0=x_sb[:, 2 * k : 2 * k + 2, :],
                    in1=g_sb[:, 2 * k : 2 * k + 2, :],
                    op=mybir.AluOpType.subtract,
                ).then_inc(vs, 1)
            for j in range(2):  # chunks 14, 15 singly into slot 2
                vector.wait_ge(xs[1], 16)
                vector.wait_ge(gt[j], 16)
                vector.tensor_tensor(
                    out=diff_sb[:, 2, j : j + 1, :],
                    in0=x_sb[:, 14 + j : 15 + j, :],
                    in1=g_sb[:, 14 + j : 15 + j, :],
                    op=mybir.AluOpType.subtract,
                ).then_inc(vs, 1)

        @block.scalar
        def _(scalar):
            for k in range(NFULL):
                scalar.wait_ge(vs, k + 1)
                if k:
                    # shared sq dummy: same-engine WAW, free wait for the
                    # race detector
                    scalar.wait_ge(ss, k)
                scalar.activation(
                    out=sq_sb[:, :, :],
                    in_=diff_sb[:, k % 2, :, :],
                    func=mybir.ActivationFunctionType.Square,
                    accum_out=acc_sb[:, k : k + 1],
                ).then_inc(ss, 1)
                if k == NFULL - 2:
                    # early store of the first 5 columns hides most of the
                    # final DMA's completion receipt behind the last pairs.
                    # ss fires on ACTIVATION_READ_ACCUMULATOR completion, so
                    # this wait orders the store after the accum writes (the
                    # DMA trigger otherwise races the accumulator read-out).
                    scalar.wait_ge(ss, NFULL - 1)
                    scalar.dma_start(
                        out=out_d[:, : NFULL - 1], in_=acc_sb[:, : NFULL - 1]
                    ).then_inc(os_, 16)
            for j in range(2):  # chunks 14, 15 singly (short tail ops)
                scalar.wait_ge(vs, NFULL + j + 1)
                scalar.wait_ge(ss, NFULL + j)
                scalar.activation(
                    out=sq_sb[:, j : j + 1, :],
                    in_=diff_sb[:, 2, j : j + 1, :],
                    func=mybir.ActivationFunctionType.Square,
                    accum_out=acc_sb[:, NFULL + j : NFULL + j + 1],
                ).then_inc(ss, 1)
            scalar.wait_ge(ss, NFULL + 2)
            scalar.dma_start(
                out=out_d[:, NFULL - 1 :], in_=acc_sb[:, NFULL - 1 :]
            ).then_inc(os_, 16)
            # no receipt wait: the ~8us block epilogue (fixed barrier +
            # teardown) runs after this engine retires, giving the 0.9us
            # store receipt ample slack before NEFF completion
            scalar.wait_ge(os_, 16)

    nc.finalize()
    return nc


def _get_nc():
    if "nc" not in _NC_CACHE:
        _NC_CACHE["nc"] = build_nc()
    return _NC_CACHE["nc"]


def kernel(x, labels, centers, _trace=False):
    x = np.asarray(x, dtype=np.float32).astype(ml_dtypes.bfloat16)
    centers = np.asarray(centers, dtype=np.float32).astype(ml_dtypes.bfloat16)
    labels_i = np.asarray(labels).astype(np.int32)

    in_maps = []
    for i in range(N_CORES):
        xs_ = np.ascontiguousarray(x[i * B_SHARD : (i + 1) * B_SHARD])
        ls_ = labels_i[i * B_SHARD : (i + 1) * B_SHARD]
        in_maps.append(
            {
                "x": xs_,
                # row 16p + r at [p, r]
                "labels": np.ascontiguousarray(ls_.reshape(P, NCHUNK)),
                "centers": centers,
            }
        )

    nc = _get_nc()
    res = run_bass_kernel_spmd(nc, in_maps, list(range(N_CORES)), trace=_trace)
    partials = np.stack([r["out"] for r in res.results])  # [8, 128, NCOL]
    total = np.sum(partials.astype(np.float64))
    total += B * (C - 1) * CLAMP_MIN
    loss = np.float32(total / B)
    if _trace:
        return np.asarray(loss), res
    return np.asarray(loss)
